# revision 15
# baseline (speedup 1.0000x reference)
"""Swin-style windowed attention (b=16, n=1024, 8 heads x 32, relative
position bias) for 8 Trainium2 NeuronCores, data-parallel over batch.

v5: instruction-minimized for this backend's flat ~70us-per-matmul cost
model (batched same-engine runs beat fine-grained pipelines; engine
switches cost ~40us; DMA transfers run in background at ~2.3GB/s).
  - groups = (head-group hg, head-pair pr, batch b) with FULL q=1024:
    one exp + one eb-mult of [128, 2048] per k-tile (64 of each total).
  - matmul outputs respect the one-bank limit (512 f32 cols); counts:
    qk 64 + v 64 + dots 256 + PV 256 + out-proj 32 + norm 16 = 688.
  - normalize: DVE reciprocals of the two denominator rows into
    partitions 0/32 of a persistent r2 tile, then ONE K=33 f32
    ones-matmul per q-half broadcasts both heads (f32 matmuls are
    self-loading: no ldweights), one PSUM->SBUF copy.
  - phase 3 head-packed: `on` tiles hold 4 heads x 32 dims on 128
    partitions, so 2 matmuls per 128-token block; bias added via DVE
    tensor add with a preloaded broadcast b_out tile; out-proj steps
    interleaved into the following group's kt loop via `pending`.
  - merged DMAs/copies: one [128,2048] qk copy per feature tile, one
    strided v_aug copy per token tile, one y DMA per batch (HWDGE).
"""

import dataclasses
from collections import deque

import numpy as np


def _ensure_path():
    try:
        import concourse.bass  # noqa: F401
    except ImportError:
        import sys

        for p in ("/opt/trn_rl_repo", "/root/.axon_site/_ro/trn_rl_repo"):
            if p not in sys.path:
                sys.path.insert(0, p)


_ensure_path()

import concourse.bass as bass  # noqa: E402
import concourse.tile as tile  # noqa: E402
from concourse import mybir  # noqa: E402
from concourse.bass_utils import run_bass_kernel_spmd  # noqa: E402

HEADS = 8
DH = 32
INP = 512
OUP = 512
N = 1024
B = 16
NCORES = 8
BPC = B // NCORES  # batches per core
T = BPC * N  # tokens per core
TABLE = 3969
KT = N // 128  # 8 k-tiles per batch image

F32 = mybir.dt.float32
DTYPES = {"bf16": mybir.dt.bfloat16, "f16": mybir.dt.float16, "f32": mybir.dt.float32}
DT = "f16"
Exp = mybir.ActivationFunctionType.Exp

_COMPUTE_CAP = 1


def _split_waits(nc, cap=1):
    """Split instructions with too many semaphore waits into same-engine
    NoOp chains (walrus on this build rejects >1 wait per instruction)."""
    n = 0
    for _, bb_wrap in nc.bb_map.items():
        bb = bb_wrap.bb if hasattr(bb_wrap, "bb") else bb_wrap
        new_list = []
        changed = False
        for inst in bb.instructions:
            si = inst.sync_info
            cap = _COMPUTE_CAP
            if si is not None and si.on_wait and len(si.on_wait) > cap:
                waits = list(si.on_wait)
                rest, head = waits[:-cap], waits[-cap:]
                for i in range(0, len(rest), cap):
                    nop = mybir.InstNoOp(name=f"{inst.name}_wsplit{i}")
                    nop.engine = inst.engine
                    nop.sync_info = mybir.SyncInfo(
                        on_wait=rest[i : i + cap], on_update=[]
                    )
                    nc.register_instruction(nop, overwrite=True)
                    new_list.append(nop)
                    n += 1
                inst.sync_info = mybir.SyncInfo(
                    on_wait=head, on_update=list(si.on_update)
                )
                changed = True
            new_list.append(inst)
        if changed:
            bb.instructions = new_list
    return n


def _emit_body(nc, tc, es, aps, BF):
    """One repetition of the full per-core computation."""
    from contextlib import ExitStack

    xT_d, w_d, eb_d, wpk_d, bout_d, y_d = aps

    # ---- persistent-for-this-rep pools (released last, LIFO) ---------------
    p_on = es.enter_context(tc.tile_pool(name="on", bufs=4))
    p_eb = es.enter_context(tc.tile_pool(name="eb", bufs=2))
    p_qk = es.enter_context(tc.tile_pool(name="qk", bufs=4))
    p_v = es.enter_context(tc.tile_pool(name="v", bufs=1))
    p_wp = es.enter_context(tc.tile_pool(name="wpk", bufs=1))
    on_tiles = {}

    # ---- phase 1: projections ---------------------------------------------
    with ExitStack() as ph1:
        p_x = ph1.enter_context(tc.tile_pool(name="xT", bufs=4))
        p_w = ph1.enter_context(tc.tile_pool(name="w", bufs=4))

        xT = []
        w = []
        for dm in range(4):
            t = p_x.tile([128, T], BF, tag="xT", name=f"xT{dm}")
            nc.sync.dma_start(t[:], xT_d[dm * 128 : (dm + 1) * 128, :])
            xT.append(t)
            t = p_w.tile([128, 768], BF, tag="w", name=f"w{dm}")
            nc.sync.dma_start(t[:], w_d[dm * 128 : (dm + 1) * 128, :])
            w.append(t)
        wpk = p_wp.tile([128, 1024], BF, tag="wpk")
        nc.sync.dma_start(wpk[:], wpk_d)
        bout = p_wp.tile([128, 512], F32, tag="bout")
        nc.sync.dma_start(bout[:], bout_d)

        # qT/kT feature-major: ft: 0=q h0-3, 1=q h4-7, 2=k h0-3, 3=k h4-7
        qk = [p_qk.tile([128, T], BF, tag="qk", name=f"qk{i}") for i in range(4)]
        # v token-major: per token-tile, 4 head-pairs of 97 cols:
        # [v_h(32) | ones | zeros(31) | v_h+1(32) | ones]; PV matmuls of
        # M=33 per head put denominators on psum rows 32/96.
        v_aug = p_v.tile([128, (T // 128) * 388], BF)
        nc.gpsimd.memset(v_aug[:], 0.0)
        va4 = v_aug[:].rearrange("p (t pr c) -> p t pr c", t=T // 128, pr=4)
        nc.gpsimd.memset(va4[:, :, :, 32:33], 1.0)
        nc.gpsimd.memset(va4[:, :, :, 96:97], 1.0)

        with ExitStack() as st1:
            p_ps1 = st1.enter_context(tc.tile_pool(name="ps1", bufs=2, space="PSUM"))
            for ft in range(4):
                ps = p_ps1.tile([128, T], F32, tag="ps1")
                for dm in range(4):
                    for tb in range(T // 512):
                        nc.tensor.matmul(
                            ps[:, tb * 512 : (tb + 1) * 512],
                            w[dm][:, ft * 128 : (ft + 1) * 128],
                            xT[dm][:, tb * 512 : (tb + 1) * 512],
                            start=(dm == 0),
                            stop=(dm == 3),
                        )
                nc.scalar.copy(qk[ft][:], ps[:])

        with ExitStack() as st2:
            p_psv = st2.enter_context(tc.tile_pool(name="psv", bufs=4, space="PSUM"))
            for tt in range(T // 128):
                ps = p_psv.tile([128, 256], F32, tag="psv")
                for dm in range(4):
                    nc.tensor.matmul(
                        ps[:],
                        xT[dm][:, tt * 128 : (tt + 1) * 128],
                        w[dm][:, 512:768],
                        start=(dm == 0),
                        stop=(dm == 3),
                    )
                # one strided copy per token tile:
                # dst col = pr*97 + e*64 + d, src col = pr*64 + e*32 + d
                dslice = v_aug[:, tt * 388 : (tt + 1) * 388]
                dst = dataclasses.replace(
                    dslice,
                    ap=[dslice.ap[0], [97, 4], [64, 2], [1, 32]],
                )
                srcv = ps[:].rearrange("p (pr e d) -> p pr e d", pr=4, e=2)
                nc.scalar.copy(dst, srcv)

    # ---- phase 2 (+ interleaved phase 3) -----------------------------------
    with ExitStack() as ph2:
        p_p = ph2.enter_context(tc.tile_pool(name="pexp", bufs=1))
        p_ph = ph2.enter_context(tc.tile_pool(name="phat", bufs=1))
        p_dots = ph2.enter_context(tc.tile_pool(name="dots", bufs=1, space="PSUM"))
        p_po = ph2.enter_context(tc.tile_pool(name="po", bufs=1, space="PSUM"))
        p_psy = ph2.enter_context(tc.tile_pool(name="psy", bufs=2, space="PSUM"))
        p_r = ph2.enter_context(tc.tile_pool(name="recip", bufs=1))
        p_y = ph2.enter_context(tc.tile_pool(name="ysb", bufs=1))
        p_one = ph2.enter_context(tc.tile_pool(name="ones", bufs=1))
        # ones2: [33, 64] block mask — partition 0 -> out rows 0-31,
        # partition 32 -> out rows 32-63 (memset bases must be 32-aligned)
        ones2 = p_one.tile([33, 64], F32, tag="ones2")
        nc.gpsimd.memset(ones2[:], 0.0)
        nc.gpsimd.memset(ones2[0:1, 0:32], 1.0)
        nc.gpsimd.memset(ones2[32:33, 32:64], 1.0)
        # r2: recip rows at partitions 0/32; rows 1-31 stay zero so the
        # K=33 ones-matmul reads no garbage
        r2 = p_one.tile([33, 1024], F32, tag="r2")
        nc.gpsimd.memset(r2[:], 0.0)

        pending = deque()

        def make_out_proj(b):
            """Output projection y[b] as emission steps (spread over the
            following group's kt loop when possible)."""
            ysb = p_y.tile([128, 8 * 512], F32, tag="ysb")

            def step(t8):
                psy = p_psy.tile([128, 512], F32, tag="psy")
                for hgx in range(2):
                    nc.tensor.matmul(
                        psy[:],
                        on_tiles[(hgx, b)][:, t8 * 128 : (t8 + 1) * 128],
                        wpk[:, hgx * 512 : (hgx + 1) * 512],
                        start=(hgx == 0),
                        stop=(hgx == 1),
                    )
                nc.vector.tensor_add(
                    ysb[:, t8 * 512 : (t8 + 1) * 512], psy[:], bout[:]
                )

            steps = [lambda t8=t8: step(t8) for t8 in range(8)]
            steps.append(lambda: nc.sync.dma_start(y_d[b], ysb[:]))
            return steps

        for hg in range(2):
            for pr in range(2):
                # one 4 MiB mega-tile per (hg, pr): free idx
                # (kt*2 + jj)*1024 + q   for jj = head-in-pair
                eb = p_eb.tile([128, 16384], BF, tag="eb", name=f"eb{hg}_{pr}")
                nc.sync.dma_start(eb[:], eb_d[hg, pr])
                for b in range(BPC):
                    if pr == 0 and (hg, b) not in on_tiles:
                        on_tiles[(hg, b)] = p_on.tile(
                            [128, 1024], BF, tag="on", name=f"on{hg}_{b}"
                        )
                    po = p_po.tile([128, 1024], F32, tag="po")
                    pv_q = deque()

                    def emit_pv(kt, Ph):
                        for jj in range(2):
                            for qh in range(2):
                                cb = jj * 64
                                prg = 2 * hg + pr
                                base = (b * KT + kt) * 388 + prg * 97
                                nc.tensor.matmul(
                                    po[cb : cb + 33, qh * 512 : (qh + 1) * 512],
                                    v_aug[:, base + jj * 64 : base + jj * 64 + 33],
                                    Ph[:, (jj * 2 + qh) * 512 : (jj * 2 + qh + 1) * 512],
                                    start=(kt == 0),
                                    stop=(kt == KT - 1),
                                    tile_position=(0, cb),
                                    skip_group_check=True,
                                )

                    P = p_p.tile([128, KT * 2048], BF, tag="pexp")
                    for kt in range(KT):
                        dots = p_dots.tile([128, 2048], F32, tag="dots")
                        for jj in range(2):
                            pb = (2 * pr + jj) * 32
                            for qh in range(2):
                                nc.tensor.matmul(
                                    dots[:, (jj * 2 + qh) * 512 : (jj * 2 + qh + 1) * 512],
                                    qk[2 + hg][
                                        pb : pb + 32,
                                        b * N + kt * 128 : b * N + kt * 128 + 128,
                                    ],
                                    qk[hg][pb : pb + 32, b * N + qh * 512 : b * N + qh * 512 + 512],
                                    start=True,
                                    stop=True,
                                    tile_position=(pb, 0),
                                )
                        nc.scalar.activation(
                            P[:, kt * 2048 : (kt + 1) * 2048], dots[:], Exp
                        )
                        if pending:
                            pending.popleft()()
                    Ph = p_ph.tile([128, KT * 2048], BF, tag="phat")
                    nc.vector.tensor_mul(Ph[:], P[:], eb[:])
                    for kt in range(KT):
                        emit_pv(kt, Ph[:, kt * 2048 : (kt + 1) * 2048])

                    # normalize: 1/den rows on DVE into partitions 0/32 of
                    # r2, ONE K=33 f32 ones-matmul per q-half broadcasts
                    # both heads, one copy to SBUF, mult into `on`.
                    nc.vector.reciprocal(r2[0:1, :], po[32:33, :])
                    nc.vector.reciprocal(r2[32:33, :], po[96:97, :])
                    rb = p_dots.tile([64, 1024], F32, tag="dots", name="rb")
                    for qh in range(2):
                        nc.tensor.matmul(
                            rb[:, qh * 512 : (qh + 1) * 512],
                            ones2[:],
                            r2[:, qh * 512 : (qh + 1) * 512],
                            start=True,
                            stop=True,
                        )
                    rs = p_r.tile([128, 1024], F32, tag="rbcast")
                    nc.vector.tensor_copy(rs[0:64, :], rb[:])

                    on = on_tiles[(hg, b)]
                    for jj in range(2):
                        nc.vector.tensor_mul(
                            on[(2 * pr + jj) * 32 : (2 * pr + jj + 1) * 32, :],
                            po[jj * 64 : jj * 64 + 32, :],
                            rs[jj * 32 : (jj + 1) * 32, :],
                        )

                    if hg == 1 and pr == 1:
                        pending.extend(make_out_proj(b))
        while pending:
            pending.popleft()()


def build_program(reps=1, dt=None):
    BF = DTYPES[dt or DT]
    nc = bass.Bass("TRN2", target_bir_lowering=False, debug=False, num_devices=NCORES)
    xT_d = nc.dram_tensor("xT", [INP, T], BF, kind="ExternalInput").ap()
    w_d = nc.dram_tensor("w", [INP, 768], BF, kind="ExternalInput").ap()
    eb_d = nc.dram_tensor("eb", [2, 2, 128, 16384], BF, kind="ExternalInput").ap()
    wpk_d = nc.dram_tensor("wpk", [128, 1024], BF, kind="ExternalInput").ap()
    bout_d = nc.dram_tensor("bout", [128, 512], F32, kind="ExternalInput").ap()
    y_d = nc.dram_tensor("y", [BPC, 128, 8 * 512], F32, kind="ExternalOutput").ap()
    aps = (xT_d, w_d, eb_d, wpk_d, bout_d, y_d)

    from contextlib import ExitStack

    with tile.TileContext(nc) as tc:
        for _ in range(reps):
            with ExitStack() as es:
                _emit_body(nc, tc, es, aps, BF)

    _split_waits(nc, cap=1)
    return nc


def _relative_index():
    ii, jj = np.meshgrid(np.arange(32), np.arange(32), indexing="ij")
    coords = np.stack([ii.reshape(-1), jj.reshape(-1)])
    rel = coords[:, :, None] - coords[:, None, :]
    return ((rel[0] + 31) * 63 + (rel[1] + 31)).reshape(-1)


def prepare_inputs(x, w_qkv, bias_table, w_out, b_out, dt=None):
    """Host-side prep: returns per-core in_maps."""
    import ml_dtypes

    bf16 = {"bf16": ml_dtypes.bfloat16, "f16": np.float16, "f32": np.float32}[dt or DT]
    scale = DH ** -0.5

    w = np.ascontiguousarray(w_qkv).astype(np.float32).copy()
    w[:, :256] *= scale
    w = w.astype(bf16)

    idx = np.clip(_relative_index(), 0, TABLE - 1)
    bias = bias_table[idx].reshape(N, N, HEADS).astype(np.float32)  # [q, k, h]
    ebT = np.exp(bias).transpose(1, 0, 2)  # [k, q, h]
    # mega-tile layout [hg, pr, p, (kt, jj, q)]: k = kt*128 + p,
    # h = hg*4 + pr*2 + jj
    eb = (
        ebT.reshape(KT, 128, N, 2, 2, 2)  # [kt, p, q, hg, pr, jj]
        .transpose(3, 4, 1, 0, 5, 2)  # [hg, pr, p, kt, jj, q]
        .reshape(2, 2, 128, 16384)
        .astype(bf16)
    )

    # packed out-projection weights: [128 = 4 heads x 32 dims, hgx*512 + o]
    wpk = np.zeros((128, 1024), np.float32)
    for hgx in range(2):
        for j in range(4):
            wpk[j * 32 : (j + 1) * 32, hgx * 512 : (hgx + 1) * 512] = w_out[
                (4 * hgx + j) * 32 : (4 * hgx + j + 1) * 32
            ]
    wpk = wpk.astype(bf16)
    bout = np.tile(b_out.astype(np.float32)[None, :], (128, 1))

    in_maps = []
    for c in range(NCORES):
        xc = x[c * BPC : (c + 1) * BPC].reshape(T, INP)
        xT = np.ascontiguousarray(xc.T.astype(np.float32)).astype(bf16)
        in_maps.append({"xT": xT, "w": w, "eb": eb, "wpk": wpk, "bout": bout})
    return in_maps


_NC_CACHE = {}


def kernel(x, w_qkv, bias_table, w_out, b_out):
    in_maps = prepare_inputs(x, w_qkv, bias_table, w_out, b_out)
    if 1 not in _NC_CACHE:
        _NC_CACHE[1] = build_program(reps=1)
    nc = _NC_CACHE[1]
    res = run_bass_kernel_spmd(nc, in_maps, list(range(NCORES)), trace=False)
    # y DRAM layout [b, p, t8*512+o]: token = b*1024 + t8*128 + p
    y = np.concatenate(
        [
            res.results[c]["y"]
            .reshape(BPC, 128, 8, OUP)
            .transpose(0, 2, 1, 3)
            .reshape(BPC, N, OUP)
            for c in range(NCORES)
        ],
        axis=0,
    )
    return y.astype(np.float32)


def unshard_core(yraw):
    return (
        yraw.reshape(BPC, 128, 8, OUP)
        .transpose(0, 2, 1, 3)
        .reshape(BPC, N, OUP)
        .astype(np.float32)
    )

